# revision 6
# baseline (speedup 1.0000x reference)
"""Trainium2 Bass kernel for the 8-qubit variational-circuit batch evaluator.

Math: the 256-dim product state collapses analytically.  With s_q = x_q^2,
Z_q = 1+s_q, zz_q = 1+s_q^2:

  out = C0 + C1*rsqrt(A) + C2*w*rsqrt(BB) + C3*w*x1^2*rsqrt(BB*P27)

  P27 = prod_{q=2..7} Z_q,  A = Z1*P27,  BB = Z0*zz0*Z1*zz1,  w = x0*x1,
  C0..C3 host-derived scalars from the 3 complex rotation weights.

rsqrt-EARLY form: take r_q = rsqrt(s_q + 1) for all 10 needed squares in ONE
activation op (ACT computes func(in*scale + bias); bias=1.0 gives
1/sqrt(1+s) directly), then multiply the small r-values:

  K = r2*..*r7 = rsqrt(P27),  R2 = (r0*rr0)*(r1*rr1) = rsqrt(BB),
  R1 = K*r1 = rsqrt(A)

This deletes all ten "+1" adds, moves 10 elems/row to the otherwise-idle
ACT engine, and leaves DVE ~25 elems/row in 13 instructions per chunk.

bf16 everywhere between fp32 input cast (host-side) and fp32 output write
(harness tolerance 2e-2, measured rel err ~4e-4).  DVE runs its 2x 16-bit
mode when every non-scalar operand has a unit-stride innermost dim >= 2:
squares are AoS [P,R,slots] (matches the feature-minor DRAM layout), the
activation converts AoS->SoA for free via a transposed *output* AP (the
strided side must be the write: a strided *read* measured ~2x slower), and
the product tree + fused scalar_tensor_tensor combines run SoA [P,slot,R].

w = x0*x1 runs on the Pool engine (independent of the rsqrt chain).
Coefficients are compile-time immediates (program cached per weight values),
so there is no coefficient DMA.  2 chunks of 64 rows pipeline
DMA-in -> squares(DVE) -> rsqrt(ACT) -> products(DVE) -> DMA-out; input
DMAs are issued back-to-back on the Sync queue (chunk0's descriptors drain
first so compute starts earliest).
"""

import numpy as np

import concourse.bass as bass
from concourse import mybir
from concourse.bass_utils import run_bass_kernel_spmd

N_CORES = 8
BATCH = 131072
NQ = 8
B_LOCAL = BATCH // N_CORES  # 16384
P = 128
R_TOTAL = B_LOCAL // P      # 128 rows per partition
CHUNK_ROWS = [64, 64]
CHUNK_OFF = [0, 64]
NCHUNK = len(CHUNK_ROWS)

F32 = mybir.dt.float32
BF16 = mybir.dt.bfloat16
AF = mybir.ActivationFunctionType
ALU = mybir.AluOpType


def _act_rsqrt_raw(nc, se, out, in_, bias):
    """InstActivation Rsqrt without bass's accuracy guard (validated on HW
    at <5e-5 rel over [1, 3e10]); bias is an AP so out = rsqrt(in + bias)."""
    ins = [se.lower_ap(in_), se.lower_ap(bias),
           mybir.ImmediateValue(dtype=mybir.dt.float32, value=1.0),
           mybir.ImmediateValue(dtype=mybir.dt.float32, value=0.0)]
    return se.add_instruction(mybir.InstActivation(
        name=nc.get_next_instruction_name(), func=AF.Rsqrt,
        ins=ins, outs=[se.lower_ap(out)]))


def _build_nc(coeffs):
    C0, C1, C2, C3 = [float(c) for c in coeffs]
    nc = bass.Bass()
    x = nc.declare_dram_parameter("x", [B_LOCAL, NQ], BF16, isOutput=False)
    y = nc.declare_dram_parameter("y", [B_LOCAL], F32, isOutput=True)

    xv = x.rearrange("(p r) q -> p r q", p=P)      # [128, 128, 8] bf16
    yv = y.rearrange("(p r) -> p r", p=P)          # [128, 128] f32

    import contextlib
    with contextlib.ExitStack() as ctx:
        junk = ctx.enter_context(nc.sbuf_tensor("junk", [P, 2], BF16))
        xts, sas, rts, pts, dts, wts, ots = [], [], [], [], [], [], []
        for c in range(NCHUNK):
            rc = CHUNK_ROWS[c]
            # AoS squares: [s0..s7, s0^2, s1^2]
            xts.append(ctx.enter_context(
                nc.sbuf_tensor(f"xt{c}", [P, rc, NQ], BF16)))
            sas.append(ctx.enter_context(
                nc.sbuf_tensor(f"sa{c}", [P, rc, 10], BF16)))
            # SoA rsqrt out: [r0..r7, rr0, rr1]
            rts.append(ctx.enter_context(
                nc.sbuf_tensor(f"rt{c}", [P, 10, rc], BF16)))
            # [p23, p45, p67, q0, q1]
            pts.append(ctx.enter_context(
                nc.sbuf_tensor(f"pt{c}", [P, 5, rc], BF16)))
            # [r2345, R2, K, R1, wR2, u, v, a]
            dts.append(ctx.enter_context(
                nc.sbuf_tensor(f"dt{c}", [P, 8, rc], BF16)))
            wts.append(ctx.enter_context(
                nc.sbuf_tensor(f"wt{c}", [P, 1, rc], BF16)))
            ots.append(ctx.enter_context(
                nc.sbuf_tensor(f"ot{c}", [P, 1, rc], F32)))
        s_in0 = ctx.enter_context(nc.semaphore("s_in0"))
        s_in1 = ctx.enter_context(nc.semaphore("s_in1"))
        s_sq = ctx.enter_context(nc.semaphore("s_sq"))
        s_rsq = ctx.enter_context(nc.semaphore("s_rsq"))
        s_w = ctx.enter_context(nc.semaphore("s_w"))
        s_out = ctx.enter_context(nc.semaphore("s_out"))
        s_done = ctx.enter_context(nc.semaphore("s_done"))
        s_ins = [s_in0, s_in1]
        block = ctx.enter_context(nc.Block())

        @block.sync
        def _(sync):
            for c in range(NCHUNK):
                sync.dma_start(
                    out=xts[c][:],
                    in_=xv[:, CHUNK_OFF[c]:CHUNK_OFF[c] + CHUNK_ROWS[c], :]
                ).then_inc(s_ins[c], 16)
            for c in range(NCHUNK):
                sync.wait_ge(s_out, c + 1)
                sync.dma_start(
                    out=yv[:, CHUNK_OFF[c]:CHUNK_OFF[c] + CHUNK_ROWS[c]],
                    in_=ots[c][:, 0, :]).then_inc(s_done, 16)

        @block.scalar
        def _(scalar):
            # prefetch the rsqrt ACT table during the input DMA wait
            bias1_junk = nc.const_aps.scalar_like(1.0, junk[:, 0:1])
            _act_rsqrt_raw(nc, scalar, junk[:, 1:2], junk[:, 0:1], bias1_junk)
            for c in range(NCHUNK):
                scalar.wait_ge(s_sq, c + 1)
                # contiguous read (AoS), strided write (SoA transpose view)
                in_t = sas[c][:, :, :]
                out_t = rts[c][:, :, :].rearrange("p s r -> p r s")
                bias1 = nc.const_aps.scalar_like(1.0, in_t)
                _act_rsqrt_raw(nc, scalar, out_t, in_t,
                               bias1).then_inc(s_rsq, 1)

        @block.gpsimd
        def _(gpsimd):
            # w = x0*x1 — independent of the rsqrt chain, Pool is idle
            for c in range(NCHUNK):
                gpsimd.wait_ge(s_ins[c], 16)
                gpsimd.tensor_mul(wts[c][:, 0, :], xts[c][:, :, 0],
                                  xts[c][:, :, 1]).then_inc(s_w, 1)

        @block.vector
        def _(vector):
            v = vector
            for c in range(NCHUNK):
                xt, sa = xts[c], sas[c]
                v.wait_ge(s_ins[c], 16)
                # squares (AoS, all-bf16 unit-stride => DVE fast mode)
                v.tensor_mul(sa[:, :, 0:8], xt[:, :, :], xt[:, :, :])
                v.tensor_mul(sa[:, :, 8:10], sa[:, :, 0:2],
                             sa[:, :, 0:2]).then_inc(s_sq, 1)
            for c in range(NCHUNK):
                rt, pt, dt, wt, sa, ot = (rts[c], pts[c], dts[c], wts[c],
                                          sas[c], ots[c])
                v.wait_ge(s_rsq, c + 1)
                # [p23,p45,p67] ; [q0,q1]
                v.tensor_mul(pt[:, 0:3, :], rt[:, 2:8:2, :], rt[:, 3:8:2, :])
                v.tensor_mul(pt[:, 3:5, :], rt[:, 0:2, :], rt[:, 8:10, :])
                # [r2345, R2] = [p23,q0]*[p45,q1]
                v.tensor_mul(dt[:, 0:2, :], pt[:, 0:4:3, :], pt[:, 1:5:3, :])
                # K = r2345*p67 ; R1 = K*r1
                v.tensor_mul(dt[:, 2:3, :], dt[:, 0:1, :], pt[:, 2:3, :])
                v.tensor_mul(dt[:, 3:4, :], dt[:, 2:3, :], rt[:, 1:2, :])
                # wR2 = w*R2 (w from Pool)
                v.wait_ge(s_w, c + 1)
                v.tensor_mul(dt[:, 4:5, :], wt[:, :, :], dt[:, 1:2, :])
                # u = (x1^2 * C3) * K   (x1^2 is AoS slot 1, transposed view)
                x1sq = sa[:, :, 1:2].rearrange("p r s -> p s r")
                v.scalar_tensor_tensor(dt[:, 5:6, :], x1sq, C3,
                                       dt[:, 2:3, :], ALU.mult, ALU.mult)
                # v = (u + C2) * wR2
                v.scalar_tensor_tensor(dt[:, 6:7, :], dt[:, 5:6, :], C2,
                                       dt[:, 4:5, :], ALU.add, ALU.mult)
                # a = (R1 * C1) + v
                v.scalar_tensor_tensor(dt[:, 7:8, :], dt[:, 3:4, :], C1,
                                       dt[:, 6:7, :], ALU.mult, ALU.add)
                # out = a + C0  (fp32 write)
                v.tensor_scalar(ot[:, 0:1, :], dt[:, 7:8, :], C0, None,
                                ALU.add).then_inc(s_out, 1)

    return nc


_NC_CACHE = {}


def _get_nc(coeffs):
    key = tuple(np.asarray(coeffs, np.float32).tolist())
    if key not in _NC_CACHE:
        _NC_CACHE[key] = _build_nc(key)
    return _NC_CACHE[key]


def _host_coeffs(weights_re, weights_im):
    w = (np.asarray(weights_re, np.float64)
         + 1j * np.asarray(weights_im, np.float64)) * 0.5
    c, s = np.cos(w), np.sin(w)

    def rymat(i):
        return np.array([[c[i], -s[i]], [s[i], c[i]]])

    rot = rymat(2) @ (rymat(1) @ rymat(0))
    A, B = rot[0, 0], rot[0, 1]
    alpha = abs(B) ** 2
    beta = abs(A) ** 2 - abs(B) ** 2
    gam = A * np.conj(B)
    return np.array([alpha + beta / 2, beta / 2, gam.real, gam.imag],
                    dtype=np.float32)


def _to_bf16(x):
    try:
        import ml_dtypes
        return x.astype(ml_dtypes.bfloat16)
    except ImportError:
        import numpy as _np
        u = x.view(_np.uint32)
        rounded = ((u + 0x7FFF + ((u >> 16) & 1)) >> 16).astype(_np.uint16)
        return rounded


def kernel(inputs, weights_re, weights_im):
    x = np.ascontiguousarray(np.asarray(inputs, dtype=np.float32))
    xb = _to_bf16(x)
    co = _host_coeffs(weights_re, weights_im)
    nc = _get_nc(co)
    shards = np.split(xb, N_CORES, axis=0)
    in_maps = [{"x": sh} for sh in shards]
    res = run_bass_kernel_spmd(nc, in_maps, list(range(N_CORES)))
    return np.concatenate([res.results[i]["y"] for i in range(N_CORES)])
